# revision 1
# baseline (speedup 1.0000x reference)
"""GATv2 message-passing network on 8 Trainium2 NeuronCores.

Sharding: 4 graphs x 2 destination-node halves. Each core owns nodes
[hf*10000, (hf+1)*10000) of graph g and processes the edges whose dst lands in
its half (sorted by dst, grouped into 128-node blocks, padded to a uniform
tile count T per block). conv1 runs edge-parallel per core; a pairwise
AllGather shares the conv1 output halves; pass 2 (conv2 + skip_conv + lin +
LayerNorm) runs on the same edge partition and writes each core's node half.

Softmax is computed without the max-subtraction (mathematically identical for
the attention ratio; scores here are O(10) so fp32 exp is safe).
"""

import os

import numpy as np

import concourse.bacc as bacc
import concourse.mybir as mybir
import concourse.tile as tile
from concourse import bass_utils

F32 = mybir.dt.float32
I16 = mybir.dt.int16

B, N, E, F, ED = 4, 20000, 640000, 64, 16
H1, HID, HC = 8, 16, 128
NCORES, HALF = 8, 10000
P = 128
NBLK = -(-HALF // P)              # 79
NEG_SLOPE = 0.2
LN_EPS = 1e-5
GCHUNK = 1024                     # edges per dma_gather call
TPG = GCHUNK // P                 # 16 tiles per gather chunk
NCHN = 500                        # nodes per chunk in the table-build phase
SUB = 125                         # node sub-tile (lhsT free dim)

_CACHE = {}


# ----------------------------------------------------------------------------
# host-side preprocessing
# ----------------------------------------------------------------------------

def _prep_cores(inputs):
    x = np.asarray(inputs["x"], np.float32)
    ea = np.asarray(inputs["edge_attr"], np.float32)
    ei = np.asarray(inputs["edge_index"], np.int64)

    cores = []
    for g in range(B):
        dst = ei[g, 1]
        for hf in range(2):
            n0 = hf * HALF
            sel = np.nonzero((dst >= n0) & (dst < n0 + HALF))[0]
            dloc = (dst[sel] - n0).astype(np.int64)
            order = np.argsort(dloc, kind="stable")
            cores.append((g, hf, sel[order], dloc[order]))

    T = 1
    counts_per_core = []
    for g, hf, e_sorted, d_sorted in cores:
        counts = np.bincount(d_sorted >> 7, minlength=NBLK)
        counts_per_core.append(counts)
        T = max(T, int(-(-counts.max() // P)))
    NT = -(-(NBLK * T) // TPG) * TPG
    NE = NT * P

    maps = []
    for (g, hf, e_sorted, d_sorted), counts in zip(cores, counts_per_core):
        src_s = ei[g, 0][e_sorted]
        dst_s = ei[g, 1][e_sorted]

        src_pad = np.zeros(NE, np.int64)
        dst_pad = np.zeros(NE, np.int64)
        dloc_pad = np.full(NE, -1.0, np.float32)
        ea_pad = np.zeros((NE, ED), np.float32)

        pos = 0
        for b in range(NBLK):
            c = int(counts[b])
            o = b * T * P
            src_pad[o:o + c] = src_s[pos:pos + c]
            dst_pad[o:o + c] = dst_s[pos:pos + c]
            dloc_pad[o:o + c] = (d_sorted[pos:pos + c] - b * P).astype(np.float32)
            ea_pad[o:o + c] = ea[g, e_sorted[pos:pos + c]]
            pos += c

        def wrap16(a):
            w = a.reshape(NE // 16, 16).T.astype(np.int16)
            return np.ascontiguousarray(np.tile(w, (8, 1)))

        eaT = np.ascontiguousarray(ea_pad.T)          # [16, NE]

        xl1 = x[g] @ np.asarray(inputs["c1_Wl"], np.float32)
        xr1 = x[g] @ np.asarray(inputs["c1_Wr"], np.float32)

        maps.append({
            "src16": wrap16(src_pad),
            "dst16": wrap16(dst_pad),
            "dloc": np.ascontiguousarray(dloc_pad.reshape(NT, P).T),
            "eaT": eaT,
            "xl1": np.ascontiguousarray(xl1),
            "xr1": np.ascontiguousarray(xr1),
            "xl1T": np.ascontiguousarray(xl1.T),
        })

    we1_np = np.asarray(inputs["c1_We"], np.float32)
    we2_np = np.concatenate([np.asarray(inputs["c2_We"], np.float32),
                             np.asarray(inputs["s_We"], np.float32)], axis=1)
    we1_4 = np.zeros((P, HC), np.float32)
    we2_4 = np.zeros((P, 32), np.float32)
    for r in (0, 64):
        we1_4[r:r + ED] = we1_np
        we2_4[r:r + ED] = we2_np

    shared = {
        "we1": np.ascontiguousarray(we1_4),
        "attb": np.ascontiguousarray(
            np.tile(np.asarray(inputs["c1_att"], np.float32).reshape(1, HC),
                    (P, 1))),
        "wsrc2": np.ascontiguousarray(np.concatenate(
            [np.asarray(inputs["c2_Wl"], np.float32),
             np.asarray(inputs["s_Wl"], np.float32)], axis=1)),
        "wdst2": np.ascontiguousarray(np.concatenate(
            [np.asarray(inputs["c2_Wr"], np.float32),
             np.asarray(inputs["s_Wr"], np.float32)], axis=1)),
        "we2": np.ascontiguousarray(we2_4),
        "att2b": np.ascontiguousarray(np.tile(np.concatenate(
            [np.asarray(inputs["c2_att"], np.float32).ravel(),
             np.asarray(inputs["s_att"], np.float32).ravel()]).reshape(1, 32),
            (P, 1))),
        "linT": np.ascontiguousarray(
            np.asarray(inputs["lin_W"], np.float32).T),
        "b1col": np.ascontiguousarray(
            np.asarray(inputs["c1_b"], np.float32).reshape(HC, 1)),
        "cby": np.ascontiguousarray(
            (np.asarray(inputs["c2_b"], np.float32)
             + np.asarray(inputs["s_b"], np.float32)
             @ np.asarray(inputs["lin_W"], np.float32).T
             + np.asarray(inputs["lin_b"], np.float32)).reshape(HID, 1)),
        "lngb": np.ascontiguousarray(np.tile(
            np.asarray(inputs["ln_g"], np.float32).reshape(1, HID), (P, 1))),
        "lnbb": np.ascontiguousarray(np.tile(
            np.asarray(inputs["ln_b"], np.float32).reshape(1, HID), (P, 1))),
        "iota": np.ascontiguousarray(
            np.tile(np.arange(P, dtype=np.float32)[None], (P, 1))),
        "ident": np.eye(P, dtype=np.float32),
        "b8": np.ascontiguousarray(
            (np.arange(HC)[None, :] // HID == np.arange(H1)[:, None])
            .astype(np.float32)),
        "b2": np.ascontiguousarray(
            (np.arange(48)[None, :] // HID == 2 * np.arange(2)[:, None])
            .astype(np.float32)),
    }
    for m in maps:
        m.update(shared)
    return maps, NT, T


# ----------------------------------------------------------------------------
# numpy simulation of the sharded algorithm (validates host prep + layout)
# ----------------------------------------------------------------------------

def numpy_sim(inputs):
    maps, NT, T = _prep_cores(inputs)
    NE = NT * P

    def unwrap16(w):
        return w[:16].T.reshape(NE).astype(np.int64)

    def unpack_ea(eaT):
        return eaT.T

    def segsum(vals, ex, dloc, width):
        tile_blk = np.repeat(np.arange(NT) // T, P)
        node = tile_blk * P + np.where(dloc >= 0, dloc, 0).astype(np.int64)
        valid = (dloc >= 0) & (tile_blk < NBLK)
        node = np.where(valid, node, NBLK * P)
        numer = np.zeros((NBLK * P + 1, vals.shape[1]), np.float32)
        den = np.zeros((NBLK * P + 1, ex.shape[1]), np.float32)
        np.add.at(numer, node, vals)
        np.add.at(den, node, ex)
        rep = np.repeat(den[:NBLK * P], width, 1) + 1e-16
        return numer[:NBLK * P] / rep, den[:NBLK * P]

    h_halves = []
    for m in maps:
        src, dst = unwrap16(m["src16"]), unwrap16(m["dst16"])
        dloc = m["dloc"].T.reshape(NE)
        ea = unpack_ea(m["eaT"])
        u = m["xl1"][src] + m["xr1"][dst] + ea @ m["we1"][:ED]
        gv = np.where(u > 0, u, NEG_SLOPE * u)
        score = (gv * m["attb"][0]).reshape(NE, H1, HID).sum(-1)
        ex = np.exp(score)
        h, _ = segsum(m["xl1"][src] * np.repeat(ex, HID, 1), ex, dloc, HID)
        h_halves.append(h[:HALF])

    outs = []
    for core, m in enumerate(maps):
        g = core // 2
        h_full = np.concatenate([h_halves[2 * g], h_halves[2 * g + 1]], 0)
        hx = h_full + m["b1col"].ravel()
        hx = np.where(hx > 0, hx, np.exp(np.minimum(hx, 0)) - 1)
        hxs = hx + m["xl1"]
        srctab = np.concatenate(
            [hx @ m["wsrc2"][:, :16], hxs @ m["wsrc2"][:, 16:]], 1)
        dsttab = np.concatenate(
            [hx @ m["wdst2"][:, :16], hxs @ m["wdst2"][:, 16:]], 1)

        src, dst = unwrap16(m["src16"]), unwrap16(m["dst16"])
        dloc = m["dloc"].T.reshape(NE)
        ea = unpack_ea(m["eaT"])
        u2 = srctab[src] + dsttab[dst] + ea @ m["we2"][:ED]
        g2 = np.where(u2 > 0, u2, NEG_SLOPE * u2)
        sc2 = (g2 * m["att2b"][0]).reshape(NE, 2, HID).sum(-1)
        ex2 = np.exp(sc2)
        a, _ = segsum(srctab[src] * np.repeat(ex2, HID, 1), ex2, dloc, HID)
        x1, xs = a[:, :16], a[:, 16:]
        yb = x1 + xs @ m["linT"] + m["cby"].ravel()
        mu = yb.mean(-1, keepdims=True)
        var = ((yb - mu) ** 2).mean(-1, keepdims=True)
        y = (yb - mu) / np.sqrt(var + LN_EPS) * m["lngb"][0] + m["lnbb"][0]
        outs.append(y[:HALF])

    return np.stack([np.concatenate([outs[2 * g], outs[2 * g + 1]], 0)
                     for g in range(B)])


# ----------------------------------------------------------------------------
# bass kernel
# ----------------------------------------------------------------------------

def _build(NT, T, stages=3, nch_limit=None):
    nc = bacc.Bacc("TRN2", target_bir_lowering=False, debug=False,
                   num_devices=NCORES)
    NE = NT * P
    AF = mybir.ActivationFunctionType
    OP = mybir.AluOpType
    AX = mybir.AxisListType

    def din(name, shape, dtype=F32):
        return nc.dram_tensor(name, list(shape), dtype, kind="ExternalInput")

    src16 = din("src16", [P, NE // 16], I16)
    dst16 = din("dst16", [P, NE // 16], I16)
    dloc_d = din("dloc", [P, NT])
    eaT_d = din("eaT", [ED, NE])
    xl1_d = din("xl1", [N, HC])
    xr1_d = din("xr1", [N, HC])
    xl1T_d = din("xl1T", [HC, N])
    we1_d = din("we1", [P, HC])
    attb_d = din("attb", [P, HC])
    wsrc2_d = din("wsrc2", [HC, 32])
    wdst2_d = din("wdst2", [HC, 32])
    we2_d = din("we2", [P, 32])
    att2b_d = din("att2b", [P, 32])
    linT_d = din("linT", [HID, HID])
    b1col_d = din("b1col", [HC, 1])
    cby_d = din("cby", [HID, 1])
    lngb_d = din("lngb", [P, HID])
    lnbb_d = din("lnbb", [P, HID])
    iota_d = din("iota", [P, P])
    ident_d = din("ident", [P, P])
    b8_d = din("b8", [H1, HC])
    b2_d = din("b2", [2, 48])

    if stages == 1:
        hdbg = nc.dram_tensor("hdbg", [HC, HALF], F32, kind="ExternalOutput")
    elif stages == 2:
        sdbg = nc.dram_tensor("sdbg", [N, 64], F32, kind="ExternalOutput")
        ddbg = nc.dram_tensor("ddbg", [N, 64], F32, kind="ExternalOutput")
    else:
        y_out = nc.dram_tensor("y", [HALF, HID], F32, kind="ExternalOutput")

    h_mine = nc.dram_tensor("h_mine", [HC, HALF], F32, kind="Internal")
    h_pair = nc.dram_tensor("h_pair", [2, HC, HALF], F32, kind="Internal")
    srctab = nc.dram_tensor("srctab", [N, 64], F32, kind="Internal")
    dsttab = nc.dram_tensor("dsttab", [N, 64], F32, kind="Internal")

    NCH = NT // TPG

    class _StopBuildT(Exception):
        pass
    global _StopBuild
    _StopBuild = _StopBuildT

    with tile.TileContext(nc) as tc:
        with tc.tile_pool(name="const", bufs=1) as cp:
            c_iota = cp.tile([P, P], F32)
            nc.sync.dma_start(c_iota[:], iota_d[:])
            c_id = cp.tile([P, P], F32)
            nc.sync.dma_start(c_id[:], ident_d[:])
            c_dloc = cp.tile([P, NT], F32)
            nc.sync.dma_start(c_dloc[:], dloc_d[:])
            c_we1 = cp.tile([P, HC], F32)
            nc.sync.dma_start(c_we1[:], we1_d[:])
            c_attb = cp.tile([P, HC], F32)
            nc.sync.dma_start(c_attb[:], attb_d[:])
            c_b8 = cp.tile([H1, HC], F32)
            nc.sync.dma_start(c_b8[:], b8_d[:])
            hT = cp.tile([P, NBLK * P], F32)
            if nch_limit is not None:
                nc.vector.memset(hT[:], 0.0)

            # ================= conv1 edge sweep =================
            with (
                tc.tile_pool(name="gat", bufs=3) as gp,
                tc.tile_pool(name="ed", bufs=4) as ep,
                tc.tile_pool(name="ps_u", bufs=2, space="PSUM") as pu_p,
                tc.tile_pool(name="ps_acc", bufs=2, space="PSUM") as pa_p,
                tc.tile_pool(name="ps_fl", bufs=1, space="PSUM") as pf_p,
            ):
                pnum = pden = None
                for ch in range(NCH if nch_limit is None else nch_limit):
                    e0 = ch * GCHUNK
                    sidx = gp.tile([P, GCHUNK // 16], I16, tag="sidx")
                    nc.sync.dma_start(
                        sidx[:], src16[:, e0 // 16:(e0 + GCHUNK) // 16])
                    didx = gp.tile([P, GCHUNK // 16], I16, tag="didx")
                    nc.sync.dma_start(
                        didx[:], dst16[:, e0 // 16:(e0 + GCHUNK) // 16])
                    xls_c = gp.tile([P, TPG, HC], F32, tag="xls")
                    nc.gpsimd.dma_gather(
                        out_ap=xls_c[:], in_ap=xl1_d[:], idxs_ap=sidx[:],
                        num_idxs=GCHUNK, num_idxs_reg=GCHUNK, elem_size=HC,
                        single_packet=True)
                    xrd_c = gp.tile([P, TPG, HC], F32, tag="xrd")
                    nc.gpsimd.dma_gather(
                        out_ap=xrd_c[:], in_ap=xr1_d[:], idxs_ap=didx[:],
                        num_idxs=GCHUNK, num_idxs_reg=GCHUNK, elem_size=HC,
                        single_packet=True)
                    ea_c = gp.tile([P, GCHUNK // 2], F32, tag="ea")
                    ea_src = eaT_d[:, e0:e0 + GCHUNK].rearrange(
                        "d (t c) -> d t c", c=P)
                    nc.sync.dma_start(
                        ea_c[0:ED, :].rearrange("d (t c) -> d t c", c=P),
                        ea_src[:, 0::2, :])
                    nc.sync.dma_start(
                        ea_c[64:64 + ED, :].rearrange("d (t c) -> d t c", c=P),
                        ea_src[:, 1::2, :])

                    for t in range(TPG):
                        gt = ch * TPG + t
                        b, k = gt // T, gt % T
                        if b >= NBLK:
                            break
                        pu = pu_p.tile([P, HC], F32, tag="pu", space="PSUM")
                        r0 = 64 * (t % 2)
                        ea_lhsT = ea_c[r0:r0 + ED,
                                       (t // 2) * P:(t // 2) * P + P]
                        nc.tensor.matmul(pu[:], lhsT=ea_lhsT,
                                         rhs=c_we1[r0:r0 + ED, :],
                                         start=True, stop=False)
                        nc.tensor.matmul(pu[:], lhsT=c_id[:],
                                         rhs=xls_c[:, t, :],
                                         start=False, stop=False)
                        nc.tensor.matmul(pu[:], lhsT=c_id[:],
                                         rhs=xrd_c[:, t, :],
                                         start=False, stop=True)
                        gl = ep.tile([P, HC], F32, tag="g")
                        nc.scalar.activation(gl[:], pu[:], AF.Prelu,
                                             alpha=NEG_SLOPE)
                        gm = ep.tile([P, HC], F32, tag="gm")
                        nc.vector.tensor_tensor(out=gm[:], in0=gl[:],
                                                in1=c_attb[:], op=OP.mult)
                        sc = ep.tile([P, H1], F32, tag="sc")
                        nc.vector.tensor_reduce(
                            out=sc[:],
                            in_=gm[:].rearrange("p (h c) -> p h c", h=H1),
                            axis=AX.X, op=OP.add)
                        ex = ep.tile([P, H1], F32, tag="ex")
                        nc.scalar.activation(ex[:], sc[:], AF.Exp)
                        exxl = ep.tile([P, HC], F32, tag="exxl")
                        nc.vector.tensor_tensor(
                            out=exxl[:].rearrange("p (h c) -> p h c", h=H1),
                            in0=xls_c[:, t, :].rearrange(
                                "p (h c) -> p h c", h=H1),
                            in1=ex[:].to_broadcast([P, H1, HID]), op=OP.mult)
                        oh = ep.tile([P, P], F32, tag="oh")
                        _oh_eng = (nc.gpsimd if os.environ.get("K_OH") == "g"
                                   else nc.vector)
                        _oh_eng.tensor_scalar(
                            out=oh[:], in0=c_iota[:],
                            scalar1=c_dloc[:, gt:gt + 1], scalar2=None,
                            op0=OP.is_equal)
                        if k == 0:
                            pnum = pa_p.tile([P, P], F32, tag="pnum",
                                             space="PSUM")
                            pden = pa_p.tile([H1, P], F32, tag="pden",
                                             space="PSUM")
                        nc.tensor.matmul(pnum[:], lhsT=exxl[:], rhs=oh[:],
                                         start=(k == 0), stop=(k == T - 1))
                        nc.tensor.matmul(pden[:], lhsT=ex[:], rhs=oh[:],
                                         start=(k == 0), stop=(k == T - 1))
                        if k == T - 1:
                            dn = ep.tile([H1, P], F32, tag="dn")
                            nc.vector.tensor_scalar(
                                out=dn[:], in0=pden[:], scalar1=1e-16,
                                scalar2=None, op0=OP.add)
                            rdn = ep.tile([H1, P], F32, tag="rdn")
                            nc.vector.reciprocal(rdn[:], dn[:])
                            pdb = pf_p.tile([P, P], F32, tag="pdb",
                                            space="PSUM")
                            nc.tensor.matmul(pdb[:], lhsT=c_b8[:], rhs=rdn[:],
                                             start=True, stop=True)
                            dnb = ep.tile([P, P], F32, tag="dnb")
                            nc.vector.tensor_copy(dnb[:], pdb[:])
                            nc.vector.tensor_tensor(
                                out=hT[:, b * P:(b + 1) * P], in0=pnum[:],
                                in1=dnb[:], op=OP.mult)

            if stages == 1:
                nc.sync.dma_start(hdbg[:], hT[:, :HALF])
            if stages >= 2:
                nc.sync.dma_start(h_mine[:], hT[:, :HALF])
            if stages >= 2:
                nc.gpsimd.collective_compute(
                    "AllGather", mybir.AluOpType.bypass,
                    replica_groups=[[0, 1], [2, 3], [4, 5], [6, 7]],
                    ins=[h_mine.ap().opt()], outs=[h_pair.ap().opt()])

            if stages >= 2:
                # ================= node phase: pass-2 tables =================
                c_b1 = cp.tile([HC, 1], F32)
                nc.sync.dma_start(c_b1[:], b1col_d[:])
                c_ws = cp.tile([HC, 32], F32)
                nc.sync.dma_start(c_ws[:], wsrc2_d[:])
                c_wd = cp.tile([HC, 32], F32)
                nc.sync.dma_start(c_wd[:], wdst2_d[:])

                with (
                    tc.tile_pool(name="nod", bufs=3) as npo,
                    tc.tile_pool(name="ps_tab", bufs=2, space="PSUM") as pt_p,
                ):
                    for cn in range(N // NCHN):
                        n0 = cn * NCHN
                        hf, off = n0 // HALF, n0 % HALF
                        hch = npo.tile([HC, NCHN], F32, tag="hch")
                        nc.sync.dma_start(
                            hch[:], h_pair[hf, :, off:off + NCHN])
                        skc = npo.tile([HC, NCHN], F32, tag="skc")
                        nc.sync.dma_start(skc[:], xl1T_d[:, n0:n0 + NCHN])
                        tb = npo.tile([HC, NCHN], F32, tag="tb")
                        nc.scalar.activation(tb[:], hch[:], AF.Identity,
                                             bias=c_b1[:, 0:1])
                        xm = npo.tile([HC, NCHN], F32, tag="xm")
                        nc.vector.tensor_scalar(out=xm[:], in0=tb[:], scalar1=0.0,
                                                scalar2=None, op0=OP.min)
                        em = npo.tile([HC, NCHN], F32, tag="em")
                        nc.scalar.activation(em[:], xm[:], AF.Exp)
                        rl = npo.tile([HC, NCHN], F32, tag="rl")
                        nc.vector.tensor_scalar(out=rl[:], in0=tb[:], scalar1=0.0,
                                                scalar2=None, op0=OP.max)
                        s1 = npo.tile([HC, NCHN], F32, tag="s1")
                        nc.vector.tensor_tensor(out=s1[:], in0=em[:], in1=rl[:],
                                                op=OP.add)
                        hx = npo.tile([HC, NCHN], F32, tag="hx")
                        nc.vector.tensor_scalar(out=hx[:], in0=s1[:], scalar1=-1.0,
                                                scalar2=None, op0=OP.add)
                        hxs = npo.tile([HC, NCHN], F32, tag="hxs")
                        nc.vector.tensor_tensor(out=hxs[:], in0=hx[:], in1=skc[:],
                                                op=OP.add)
                        stg_s = npo.tile([SUB, NCHN // SUB, 32], F32, tag="stg_s")
                        stg_d = npo.tile([SUB, NCHN // SUB, 32], F32, tag="stg_d")
                        for j in range(NCHN // SUB):
                            sl = slice(j * SUB, (j + 1) * SUB)
                            pts = pt_p.tile([SUB, 32], F32, tag="pts",
                                            space="PSUM")
                            nc.tensor.matmul(pts[:, 0:16], lhsT=hx[:, sl],
                                             rhs=c_ws[:, 0:16], start=True,
                                             stop=True)
                            nc.tensor.matmul(pts[:, 16:32], lhsT=hxs[:, sl],
                                             rhs=c_ws[:, 16:32], start=True,
                                             stop=True)
                            nc.vector.tensor_copy(stg_s[:, j, :], pts[:])
                            ptd = pt_p.tile([SUB, 32], F32, tag="ptd",
                                            space="PSUM")
                            nc.tensor.matmul(ptd[:, 0:16], lhsT=hx[:, sl],
                                             rhs=c_wd[:, 0:16], start=True,
                                             stop=True)
                            nc.tensor.matmul(ptd[:, 16:32], lhsT=hxs[:, sl],
                                             rhs=c_wd[:, 16:32], start=True,
                                             stop=True)
                            nc.vector.tensor_copy(stg_d[:, j, :], ptd[:])
                        nc.sync.dma_start(
                            srctab[n0:n0 + NCHN, 0:32].rearrange(
                                "(j p) c -> p j c", p=SUB), stg_s[:])
                        nc.sync.dma_start(
                            dsttab[n0:n0 + NCHN, 0:32].rearrange(
                                "(j p) c -> p j c", p=SUB), stg_d[:])

            if stages == 2:
                nc.sync.dma_start(sdbg[:, 0:32], srctab[:, 0:32])
                nc.sync.dma_start(ddbg[:, 0:32], dsttab[:, 0:32])
            if stages >= 3:
                # ================= pass-2 edge sweep =================
                c_we2 = cp.tile([P, 32], F32)
                nc.sync.dma_start(c_we2[:], we2_d[:])
                c_att2 = cp.tile([P, 32], F32)
                nc.sync.dma_start(c_att2[:], att2b_d[:])
                c_b2 = cp.tile([2, 48], F32)
                nc.sync.dma_start(c_b2[:], b2_d[:])
                c_linT = cp.tile([HID, HID], F32)
                nc.sync.dma_start(c_linT[:], linT_d[:])
                c_cby = cp.tile([HID, 1], F32)
                nc.sync.dma_start(c_cby[:], cby_d[:])
                c_lng = cp.tile([P, HID], F32)
                nc.sync.dma_start(c_lng[:], lngb_d[:])
                c_lnb = cp.tile([P, HID], F32)
                nc.sync.dma_start(c_lnb[:], lnbb_d[:])

                with (
                    tc.tile_pool(name="gat2", bufs=3) as gp2,
                    tc.tile_pool(name="ed2", bufs=4) as ep2,
                    tc.tile_pool(name="ps_u2", bufs=2, space="PSUM") as pu2_p,
                    tc.tile_pool(name="ps_ac2", bufs=2, space="PSUM") as pa2_p,
                    tc.tile_pool(name="ps_fl2", bufs=1, space="PSUM") as pf2_p,
                ):
                    pagg = pden2 = None
                    for ch in range(NCH):
                        e0 = ch * GCHUNK
                        sidx = gp2.tile([P, GCHUNK // 16], I16, tag="sidx2")
                        nc.sync.dma_start(
                            sidx[:], src16[:, e0 // 16:(e0 + GCHUNK) // 16])
                        didx = gp2.tile([P, GCHUNK // 16], I16, tag="didx2")
                        nc.sync.dma_start(
                            didx[:], dst16[:, e0 // 16:(e0 + GCHUNK) // 16])
                        srcg = gp2.tile([P, TPG, 64], F32, tag="srcg")
                        nc.gpsimd.dma_gather(
                            out_ap=srcg[:], in_ap=srctab[:], idxs_ap=sidx[:],
                            num_idxs=GCHUNK, num_idxs_reg=GCHUNK, elem_size=64,
                            single_packet=True)
                        dstg = gp2.tile([P, TPG, 64], F32, tag="dstg")
                        nc.gpsimd.dma_gather(
                            out_ap=dstg[:], in_ap=dsttab[:], idxs_ap=didx[:],
                            num_idxs=GCHUNK, num_idxs_reg=GCHUNK, elem_size=64,
                            single_packet=True)
                        ea_c = gp2.tile([P, GCHUNK // 2], F32, tag="ea2")
                        ea_src = eaT_d[:, e0:e0 + GCHUNK].rearrange(
                            "d (t c) -> d t c", c=P)
                        nc.sync.dma_start(
                            ea_c[0:ED, :].rearrange("d (t c) -> d t c", c=P),
                            ea_src[:, 0::2, :])
                        nc.sync.dma_start(
                            ea_c[64:64 + ED, :].rearrange("d (t c) -> d t c", c=P),
                            ea_src[:, 1::2, :])

                        for t in range(TPG):
                            gt = ch * TPG + t
                            b, k = gt // T, gt % T
                            if b >= NBLK:
                                break
                            pu2 = pu2_p.tile([P, 32], F32, tag="pu2",
                                             space="PSUM")
                            r0 = 64 * (t % 2)
                            ea_lhsT = ea_c[r0:r0 + ED,
                                           (t // 2) * P:(t // 2) * P + P]
                            nc.tensor.matmul(pu2[:], lhsT=ea_lhsT,
                                             rhs=c_we2[r0:r0 + ED, :],
                                             start=True, stop=False)
                            nc.tensor.matmul(pu2[:], lhsT=c_id[:],
                                             rhs=srcg[:, t, 0:32],
                                             start=False, stop=False)
                            nc.tensor.matmul(pu2[:], lhsT=c_id[:],
                                             rhs=dstg[:, t, 0:32],
                                             start=False, stop=True)
                            g2 = ep2.tile([P, 32], F32, tag="g2")
                            nc.scalar.activation(g2[:], pu2[:], AF.Prelu,
                                                 alpha=NEG_SLOPE)
                            gm2 = ep2.tile([P, 32], F32, tag="gm2")
                            nc.vector.tensor_tensor(out=gm2[:], in0=g2[:],
                                                    in1=c_att2[:], op=OP.mult)
                            sc2 = ep2.tile([P, 2], F32, tag="sc2")
                            nc.vector.tensor_reduce(
                                out=sc2[:],
                                in_=gm2[:].rearrange("p (h c) -> p h c", h=2),
                                axis=AX.X, op=OP.add)
                            ex2 = ep2.tile([P, 2], F32, tag="ex2")
                            nc.scalar.activation(ex2[:], sc2[:], AF.Exp)
                            rhs2 = ep2.tile([P, 48], F32, tag="rhs2")
                            nc.vector.tensor_tensor(
                                out=rhs2[:, 0:16],
                                in0=srcg[:, t, 0:16],
                                in1=ex2[:, 0:1].to_broadcast([P, 16]),
                                op=OP.mult)
                            nc.vector.tensor_tensor(
                                out=rhs2[:, 32:48],
                                in0=srcg[:, t, 16:32],
                                in1=ex2[:, 1:2].to_broadcast([P, 16]),
                                op=OP.mult)
                            oh = ep2.tile([P, P], F32, tag="oh2")
                            _oh_eng = (nc.gpsimd if os.environ.get("K_OH") == "g"
                                       else nc.vector)
                            _oh_eng.tensor_scalar(
                                out=oh[:], in0=c_iota[:],
                                scalar1=c_dloc[:, gt:gt + 1], scalar2=None,
                                op0=OP.is_equal)
                            if k == 0:
                                pagg = pa2_p.tile([48, P], F32, tag="pagg",
                                                  space="PSUM")
                                pden2 = pa2_p.tile([2, P], F32, tag="pden2",
                                                   space="PSUM")
                            nc.tensor.matmul(pagg[:], lhsT=rhs2[:], rhs=oh[:],
                                             start=(k == 0), stop=(k == T - 1))
                            nc.tensor.matmul(pden2[:], lhsT=ex2[:], rhs=oh[:],
                                             start=(k == 0), stop=(k == T - 1))
                            if k == T - 1:
                                dn2 = ep2.tile([2, P], F32, tag="dn2")
                                nc.vector.tensor_scalar(
                                    out=dn2[:], in0=pden2[:], scalar1=1e-16,
                                    scalar2=None, op0=OP.add)
                                rdn2 = ep2.tile([2, P], F32, tag="rdn2")
                                nc.vector.reciprocal(rdn2[:], dn2[:])
                                pdb2 = pf2_p.tile([48, P], F32, tag="fl2",
                                                  space="PSUM")
                                nc.tensor.matmul(pdb2[:], lhsT=c_b2[:],
                                                 rhs=rdn2[:], start=True,
                                                 stop=True)
                                dnb2 = ep2.tile([48, P], F32, tag="dnb2")
                                nc.vector.tensor_copy(dnb2[:], pdb2[:])
                                a1 = ep2.tile([HID, P], F32, tag="a1")
                                nc.vector.tensor_tensor(
                                    out=a1[:], in0=pagg[0:16, :],
                                    in1=dnb2[0:16, :], op=OP.mult)
                                a2 = ep2.tile([HID, P], F32, tag="a2")
                                nc.vector.tensor_tensor(
                                    out=a2[:], in0=pagg[32:48, :],
                                    in1=dnb2[32:48, :], op=OP.mult)
                                px2 = pf2_p.tile([HID, P], F32, tag="fl2",
                                                 space="PSUM")
                                nc.tensor.matmul(px2[:], lhsT=c_linT[:],
                                                 rhs=a2[:], start=True, stop=True)
                                yb = ep2.tile([HID, P], F32, tag="yb")
                                nc.vector.tensor_tensor(out=yb[:], in0=a1[:],
                                                        in1=px2[:], op=OP.add)
                                yb2 = ep2.tile([HID, P], F32, tag="yb2")
                                nc.vector.tensor_scalar(
                                    out=yb2[:], in0=yb[:],
                                    scalar1=c_cby[:, 0:1], scalar2=None,
                                    op0=OP.add)
                                pyt = pf2_p.tile([P, HID], F32, tag="fl2",
                                                 space="PSUM")
                                nc.tensor.transpose(pyt[:], yb2[:], c_id[0:HID, 0:HID])
                                s = ep2.tile([P, 1], F32, tag="s")
                                nc.vector.tensor_reduce(out=s[:], in_=pyt[:],
                                                        axis=AX.X, op=OP.add)
                                mu = ep2.tile([P, 1], F32, tag="mu")
                                nc.vector.tensor_scalar(
                                    out=mu[:], in0=s[:], scalar1=1.0 / HID,
                                    scalar2=None, op0=OP.mult)
                                cen = ep2.tile([P, HID], F32, tag="cen")
                                nc.vector.tensor_scalar(
                                    out=cen[:], in0=pyt[:], scalar1=mu[:, 0:1],
                                    scalar2=None, op0=OP.subtract)
                                sqd = ep2.tile([P, HID], F32, tag="sqd")
                                ssq = ep2.tile([P, 1], F32, tag="ssq")
                                nc.scalar.activation(sqd[:], cen[:], AF.Square,
                                                     accum_out=ssq[:])
                                vr = ep2.tile([P, 1], F32, tag="vr")
                                nc.vector.tensor_scalar(
                                    out=vr[:], in0=ssq[:], scalar1=1.0 / HID,
                                    scalar2=LN_EPS, op0=OP.mult, op1=OP.add)
                                stdv = ep2.tile([P, 1], F32, tag="stdv")
                                nc.scalar.activation(stdv[:], vr[:], AF.Sqrt)
                                rstd = ep2.tile([P, 1], F32, tag="rstd")
                                nc.vector.reciprocal(rstd[:], stdv[:])
                                yn = ep2.tile([P, HID], F32, tag="yn")
                                nc.vector.tensor_scalar(
                                    out=yn[:], in0=cen[:], scalar1=rstd[:, 0:1],
                                    scalar2=None, op0=OP.mult)
                                yg = ep2.tile([P, HID], F32, tag="yg")
                                nc.vector.tensor_tensor(out=yg[:], in0=yn[:],
                                                        in1=c_lng[:], op=OP.mult)
                                yf = ep2.tile([P, HID], F32, tag="yf")
                                nc.vector.tensor_tensor(out=yf[:], in0=yg[:],
                                                        in1=c_lnb[:], op=OP.add)
                                nrows = min(P, HALF - b * P)
                                nc.sync.dma_start(
                                    y_out[b * P:b * P + nrows, :],
                                    yf[:nrows, :])

    nc.compile()
    return nc


def kernel(**inputs):
    maps, NT, T = _prep_cores(inputs)
    key = (NT, T)
    if key not in _CACHE:
        _CACHE[key] = _build(NT, T)
    nc = _CACHE[key]
    res = bass_utils.run_bass_kernel_spmd(
        nc, maps, core_ids=list(range(NCORES)))
    outs = [res.results[c]["y"] for c in range(NCORES)]
    return np.stack([np.concatenate([outs[2 * g], outs[2 * g + 1]], 0)
                     for g in range(B)])

